# revision 38
# baseline (speedup 1.0000x reference)
"""Trainium2 Bass kernel: attention block (B=2, S=M=2048, E=1024, H=16, D=64) + LayerNorm.

Sharding: 8 cores = 2 batches x 4 query-slices of 512 rows each. Each core
computes QKV projections (K/V for the full 2048 keys of its batch), all 16
heads of attention for its 512 query rows, the output projection + residual +
LayerNorm, and the head-mean attention map for its rows. Outputs are
concatenated on the host -- no cross-core communication needed.

Device dataflow ("transposed" orientation, everything bf16 into fp32 PSUM):
  keys^T/values^T/query^T via PE matmul-transpose of cast-DMA'd bf16 chunks
  K^T[f,m] = WkT.T @ keysT      (bias via K=1 matmul)
  V[m,f]   = valuesT.T @ WvT    (+ ones column per head for softmax row-sums)
  Q^T[f,s] = WqT.T @ queryT
  scores^T[m,s] = KT_h.T @ QT_h  -> exp (scale=1/8 folded in) -> E^T bf16
  attended^T[d,s] (+rowsum row) = V_aug.T @ E^T, accumulated over m-chunks
  r = 1/rowsum; attnT = attended^T * r;  avg^T += E^T * r  (per head)
  y^T = WoT.T @ attnT (+bo);  y = transpose(y^T) + query;  LayerNorm; out
  avg = transpose(avg^T) / 16
"""

import numpy as np

B, S, M, E, H = 2, 2048, 2048, 1024, 16
D = E // H
NCORES = 8
GROUP = 4
SLOC = S // GROUP  # 512 query rows per core
P = 128
EI = E // P   # 8  contraction chunks over features
FO = E // P   # 8  output-feature chunks
MO = M // P   # 16 key chunks
SG = SLOC // P  # 4 s-chunks
VSTRIDE = 66  # per-head stride in V tile: 64 values + 1 ones + 1 pad
LN_EPS = 1e-5

_CACHE = {}


def _build_nc():
    import concourse.bass as bass
    import concourse.tile as tile
    from concourse import bacc, mybir
    from concourse.masks import make_identity

    f32 = mybir.dt.float32
    bf16 = mybir.dt.bfloat16
    f16 = mybir.dt.float16
    AF = mybir.ActivationFunctionType

    nc = bacc.Bacc(
        "TRN2", target_bir_lowering=False, debug=False, num_devices=NCORES
    )

    dq = nc.dram_tensor("q", [SLOC, E], f32, kind="ExternalInput").ap()
    dkeys = nc.dram_tensor("keys", [SLOC, E], f32, kind="ExternalInput").ap()
    dvalues = nc.dram_tensor("values", [SLOC, E], f32, kind="ExternalInput").ap()
    # DRAM staging for the K/V AllGather (4-way tensor parallel per batch).
    # K^T slice and V slice ride in ONE gather (both 2-byte dtypes).
    KCOLS = FO * SLOC            # 4096
    VCOLS = 4 * 16 * VSTRIDE     # 4224
    kt_stage = nc.dram_tensor("kt_stage", [P * KCOLS], bf16).ap()
    kt_gath = nc.dram_tensor("kt_gath", [GROUP * P * KCOLS], bf16).ap()
    v_stage = nc.dram_tensor("v_stage", [P * VCOLS], bf16).ap()
    v_gath = nc.dram_tensor("v_gath", [GROUP * P * VCOLS], bf16).ap()
    dwq = nc.dram_tensor("wq", [E, E], f32, kind="ExternalInput").ap()
    dwk = nc.dram_tensor("wk", [E, E], f32, kind="ExternalInput").ap()
    dwv = nc.dram_tensor("wv", [E, E], f32, kind="ExternalInput").ap()
    dwo = nc.dram_tensor("wo", [E, E], f32, kind="ExternalInput").ap()
    dbq = nc.dram_tensor("bq", [E], f32, kind="ExternalInput").ap()
    dbk = nc.dram_tensor("bk", [E], f32, kind="ExternalInput").ap()
    dbv = nc.dram_tensor("bv", [E], f32, kind="ExternalInput").ap()
    dbo = nc.dram_tensor("bo", [E], f32, kind="ExternalInput").ap()
    dgamma = nc.dram_tensor("gamma", [E], f32, kind="ExternalInput").ap()
    dbeta = nc.dram_tensor("beta", [E], f32, kind="ExternalInput").ap()
    dout = nc.dram_tensor("out_slice", [SLOC, E], f32, kind="ExternalOutput").ap()
    davg = nc.dram_tensor("avg_slice", [SLOC, M], f32, kind="ExternalOutput").ap()

    with tile.TileContext(nc) as tc:
        _frees = []

        def _tc_tile(shape, dtype, name):
            t, fr = tc.tile(shape, dtype, name=name)
            _frees.append(fr)
            return t

        # ---------------- constants ----------------
        ident_bf = _tc_tile([P, P], bf16, name="ident_bf")
        make_identity(nc, ident_bf)
        ident_f32 = _tc_tile([P, P], f32, name="ident_f32")
        make_identity(nc, ident_f32)
        ident_f16 = _tc_tile([P, P], f16, name="ident_f16")
        make_identity(nc, ident_f16)
        ones_r512 = _tc_tile([1, 512], bf16, name="ones_r512")
        nc.vector.memset(ones_r512, 1.0)
        ones_r128 = _tc_tile([1, P], bf16, name="ones_r128")
        nc.vector.memset(ones_r128, 1.0)
        ones_f128 = _tc_tile([1, P], f32, name="ones_f128")
        nc.vector.memset(ones_f128, 1.0)
        eps_sb = _tc_tile([P, 1], f32, name="eps_sb")
        nc.vector.memset(eps_sb, LN_EPS)

        # biases as [1, E] bf16 (loaded later, after the first weight chunks)
        bias_sb = {}
        for nm in ("bq", "bk", "bv", "bo"):
            bias_sb[nm] = _tc_tile([1, E], bf16, name=f"{nm}_sb")

        # ---------------- persistent big tiles ----------------
        KT = _tc_tile([P, FO * M], bf16, name="KT")        # [f, m]
        V4 = _tc_tile([P, MO * 16 * VSTRIDE], f16, name="V4")   # [m, h*66]
        QT = _tc_tile([P, FO * SLOC], bf16, name="QT")     # [f, s]
        WoT = _tc_tile([P, EI * E], bf16, name="WoT")     # [e_in, f]
        avg = _tc_tile([P, MO * SLOC], f16, name="avg")   # [m, s] f16 accumulator
        attnT = _tc_tile([P, EI * SLOC], bf16, name="attnT")  # [e, s]
        gamma_rep = _tc_tile([P, E], f32, name="gamma_rep")
        beta_rep = _tc_tile([P, E], f32, name="beta_rep")

        v4v = V4.rearrange("p (mi h c) -> p mi h c", mi=MO, h=16)

        # ================= phase 0/1: transposes + projections =================
        with (
            tc.tile_pool(name="ph01", bufs=1) as ph01,
            tc.tile_pool(name="stage", bufs=8) as stage,
            tc.tile_pool(name="ps01", bufs=1, space="PSUM") as ps01,
        ):
            inT = None  # [128, 16384] bf16, reused for keysT/valuesT
            wT = None   # [128, 8192] bf16, reused for WkT/WvT/WqT

            def load_transposed(dsrc, nrows, dst_tag, ident):
                """DMA [nrows, E] f32 -> bf16 chunks, PE-transpose into a
                [128, EI*nrows] bf16 tile (e on partitions)."""
                dstT = ph01.tile([P, EI * nrows], bf16, tag=dst_tag, name=dst_tag)
                nch = nrows // P
                for mi in range(nch):
                    st = stage.tile([P, E], bf16, tag="stage", name=f"st_{dst_tag}_{mi}")
                    nc.gpsimd.dma_start(out=st, in_=dsrc[mi * P:(mi + 1) * P, :])
                    pst = ps01.tile([P, E], bf16, tag="tr", bufs=3,
                                    name=f"pst_{dst_tag}_{mi}")
                    for ei in range(EI):
                        nc.tensor.transpose(
                            pst[:, ei * P:(ei + 1) * P],
                            st[:, ei * P:(ei + 1) * P],
                            ident,
                        )
                    # strided copy: block ei -> dstT cols ei*nrows + mi*128
                    dv = dstT.rearrange("p (ei m) -> p ei m", ei=EI)
                    eng = nc.vector if mi % 2 == 0 else nc.scalar
                    if eng is nc.vector:
                        nc.vector.tensor_copy(
                            dv[:, :, mi * P:(mi + 1) * P],
                            pst.rearrange("p (ei m) -> p ei m", ei=EI),
                        )
                    else:
                        nc.scalar.copy(
                            dv[:, :, mi * P:(mi + 1) * P],
                            pst.rearrange("p (ei m) -> p ei m", ei=EI),
                        )
                return dstT

            def load_w_transposed(dsrc, nm):
                wt = ph01.tile([P, EI * E], bf16, tag="wT", name=nm)
                wv_ = wt.rearrange("p (ei f) -> p ei f", ei=EI)
                for fi in range(FO):
                    st = stage.tile([P, E], bf16, tag="stage", name=f"st_{nm}_{fi}")
                    nc.gpsimd.dma_start(out=st, in_=dsrc[fi * P:(fi + 1) * P, :])
                    pst = ps01.tile([P, E], bf16, tag="tr", bufs=3,
                                    name=f"pst_{nm}_{fi}")
                    for ei in range(EI):
                        nc.tensor.transpose(
                            pst[:, ei * P:(ei + 1) * P],
                            st[:, ei * P:(ei + 1) * P],
                            ident_bf,
                        )
                    eng = nc.vector if fi % 2 == 0 else nc.scalar
                    if eng is nc.vector:
                        nc.vector.tensor_copy(
                            wv_[:, :, fi * P:(fi + 1) * P],
                            pst.rearrange("p (ei f) -> p ei f", ei=EI),
                        )
                    else:
                        nc.scalar.copy(
                            wv_[:, :, fi * P:(fi + 1) * P],
                            pst.rearrange("p (ei f) -> p ei f", ei=EI),
                        )
                return wt

            # ---- K^T slice = Wk @ keys_slice^T, then AllGather over group ----
            wT = load_w_transposed(dwk, "WkT")
            for nm, dap in (("bk", dbk), ("bv", dbv), ("bq", dbq), ("bo", dbo)):
                nc.gpsimd.dma_start(
                    out=bias_sb[nm],
                    in_=bass.AP(dap.tensor, 0, [[E, 1], [1, E]]))
            inT = load_transposed(dkeys, SLOC, "inT", ident_bf)
            wv_ = wT.rearrange("p (ei f) -> p ei f", ei=EI)
            inv = inT.rearrange("p (ei m) -> p ei m", ei=EI)
            ktv = KT.rearrange("p (fo m) -> p fo m", fo=FO)
            ktp = ph01.tile([P, FO * SLOC], bf16, tag="ktp", name="ktp")
            ktpv = ktp.rearrange("p (fo m) -> p fo m", fo=FO)
            for fo in range(FO):
                ps = ps01.tile([P, 512], f32, tag=f"proj{fo % 4}", bufs=1,
                               name=f"ps_kt_{fo}")
                for ei in range(EI):
                    nc.tensor.matmul(
                        ps,
                        lhsT=wv_[:, ei, fo * P:(fo + 1) * P],
                        rhs=inv[:, ei, :],
                        start=(ei == 0), stop=False,
                    )
                nc.tensor.matmul(
                    ps, lhsT=bias_sb["bk"][:, fo * P:(fo + 1) * P],
                    rhs=ones_r512, start=False, stop=True,
                )
                eng = nc.vector if fo % 2 == 0 else nc.scalar
                if eng is nc.vector:
                    nc.vector.tensor_copy(ktpv[:, fo, :], ps)
                else:
                    nc.scalar.copy(ktpv[:, fo, :], ps)
            nc.sync.dma_start(
                out=kt_stage.rearrange("(p n) -> p n", p=P), in_=ktp)
            nc.gpsimd.collective_compute(
                "AllGather", mybir.AluOpType.bypass,
                replica_groups=[[0, 1, 2, 3], [4, 5, 6, 7]],
                ins=[kt_stage], outs=[kt_gath],
            )


            # ---- V slice = values_slice @ Wv^T, then AllGather ----
            wT = load_w_transposed(dwv, "WvT")  # reuses wT slot
            inT = load_transposed(dvalues, SLOC, "inT", ident_bf)
            wv_ = wT.rearrange("p (ei f) -> p ei f", ei=EI)
            inv = inT.rearrange("p (ei m) -> p ei m", ei=EI)
            vp = ph01.tile([P, 4 * 16 * VSTRIDE], f16, tag="vp", name="vp")
            vpv = vp.rearrange("p (mi h c) -> p mi h c", mi=4, h=16)
            nc.vector.memset(vpv[:, :, :, 64:65], 1.0)
            for mi in range(4):
                for half in range(2):
                    ps = ps01.tile([P, 512], f32, tag=f"proj{(2 * mi + half) % 4}",
                                   bufs=1, name=f"ps_v_{mi}_{half}")
                    for ei in range(EI):
                        nc.tensor.matmul(
                            ps,
                            lhsT=inv[:, ei, mi * P:(mi + 1) * P],
                            rhs=wv_[:, ei, half * 512:(half + 1) * 512],
                            start=(ei == 0), stop=False,
                        )
                    nc.tensor.matmul(
                        ps, lhsT=ones_r128,
                        rhs=bias_sb["bv"][:, half * 512:(half + 1) * 512],
                        start=False, stop=True,
                    )
                    psv = ps.rearrange("p (h c) -> p h c", h=8)
                    eng = nc.vector if (mi * 2 + half) % 2 == 0 else nc.scalar
                    if eng is nc.vector:
                        nc.vector.tensor_copy(
                            vpv[:, mi, half * 8:(half + 1) * 8, 0:64], psv)
                    else:
                        nc.scalar.copy(
                            vpv[:, mi, half * 8:(half + 1) * 8, 0:64], psv)
            nc.sync.dma_start(
                out=v_stage.rearrange("(p n) -> p n", p=P),
                in_=vp.bitcast(bf16))
            nc.gpsimd.collective_compute(
                "AllGather", mybir.AluOpType.bypass,
                replica_groups=[[0, 1, 2, 3], [4, 5, 6, 7]],
                ins=[v_stage], outs=[v_gath],
            )


            # ---- Q^T = Wq @ query^T ----
            wT = load_w_transposed(dwq, "WqT")
            wv_ = wT.rearrange("p (ei f) -> p ei f", ei=EI)
            qT = ph01.tile([P, EI * SLOC], bf16, tag="qT", name="queryT")
            qv = qT.rearrange("p (ei s) -> p ei s", ei=EI)
            for mi in range(SG):
                st = stage.tile([P, E], bf16, tag="stage", name=f"st_q_{mi}")
                nc.gpsimd.dma_start(out=st, in_=dq[mi * P:(mi + 1) * P, :])
                pst = ps01.tile([P, E], bf16, tag="tr", bufs=3, name=f"pst_q_{mi}")
                for ei in range(EI):
                    nc.tensor.transpose(
                        pst[:, ei * P:(ei + 1) * P],
                        st[:, ei * P:(ei + 1) * P],
                        ident_bf,
                    )
                nc.vector.tensor_copy(
                    qv[:, :, mi * P:(mi + 1) * P],
                    pst.rearrange("p (ei s) -> p ei s", ei=EI),
                )
            qtv = QT.rearrange("p (fo s) -> p fo s", fo=FO)
            for fo in range(FO):
                ps = ps01.tile([P, 512], f32, tag=f"proj{fo % 4}", bufs=1,
                               name=f"ps_qt_{fo}")
                for ei in range(EI):
                    nc.tensor.matmul(
                        ps,
                        lhsT=wv_[:, ei, fo * P:(fo + 1) * P],
                        rhs=qv[:, ei, :],
                        start=(ei == 0), stop=False,
                    )
                nc.tensor.matmul(
                    ps, lhsT=bias_sb["bq"][:, fo * P:(fo + 1) * P],
                    rhs=ones_r512, start=False, stop=True,
                )
                eng = nc.vector if fo % 2 == 0 else nc.scalar
                if eng is nc.vector:
                    nc.vector.tensor_copy(qtv[:, fo, :], ps)
                else:
                    nc.scalar.copy(qtv[:, fo, :], ps)

            # ---- WoT (kept for output projection) ----
            wt_o = load_w_transposed(dwo, "WoT_tmp")
            nc.vector.tensor_copy(WoT, wt_o)

            # ---- replicate gamma/beta across partitions (overlaps gather) ----
            gb_raw = ph01.tile([1, 2 * E], f32, tag="gb", name="gb_raw")
            nc.sync.dma_start(out=gb_raw[:, 0:E],
                              in_=bass.AP(dgamma.tensor, 0, [[E, 1], [1, E]]))
            nc.sync.dma_start(out=gb_raw[:, E:2 * E],
                              in_=bass.AP(dbeta.tensor, 0, [[E, 1], [1, E]]))
            for i, dst in ((0, gamma_rep), (1, beta_rep)):
                for j in range(2):
                    ps = ps01.tile([P, 512], f32, tag=f"proj{j}", bufs=1,
                                   name=f"ps_g_{i}_{j}")
                    nc.tensor.matmul(
                        ps, lhsT=ones_f128,
                        rhs=gb_raw[:, i * E + j * 512: i * E + (j + 1) * 512],
                        start=True, stop=True,
                    )
                    nc.vector.tensor_copy(dst[:, j * 512:(j + 1) * 512], ps)

        # gather -> SBUF loads (outside ph01 so its pools release early)
        ktg3 = kt_gath.rearrange("(r p fo m) -> r p fo m", r=GROUP, p=P, fo=FO)
        for r in range(GROUP):
            nc.sync.dma_start(
                out=ktv[:, :, r * SLOC:(r + 1) * SLOC], in_=ktg3[r])
        vg = v_gath.rearrange("(r p n) -> r p n", r=GROUP, p=P)
        v4f = V4.rearrange("p (mi n) -> p mi n", mi=GROUP)
        for r in range(GROUP):
            nc.sync.dma_start(out=v4f[:, r, :], in_=vg[r].bitcast(f16))

        # ================= phase 2: attention =================
        with (
            tc.tile_pool(name="att_sb", bufs=1) as att_sb,
            tc.tile_pool(name="ps_att", bufs=1, space="PSUM") as ps_att_pool,
        ):
            avgv = avg.rearrange("p (mo s) -> p mo s", mo=MO)
            atv = attnT.rearrange("p (ei s) -> p ei s", ei=EI)
            for h in range(H):
                fo = h // 2
                poff = (h % 2) * 64
                ET = att_sb.tile([P, MO * SLOC], f16, tag="ET", bufs=3,
                                 name=f"ET_{h}")
                etv = ET.rearrange("p (mo s) -> p mo s", mo=MO)
                ps_at = ps_att_pool.tile([65, 512], f32, tag="att", bufs=2,
                                         name=f"ps_at_{h}")
                # scores^T in groups of 2 m-chunks (double-buffered psum)
                for g in range(MO // 2):
                    ps_sc = ps_att_pool.tile([P, 1024], f32, tag="sc", bufs=2,
                                             name=f"ps_sc_{h}_{g}")
                    for j in range(2):
                        mo = 2 * g + j
                        nc.tensor.matmul(
                            ps_sc[:, j * 512:(j + 1) * 512],
                            lhsT=ktv[poff:poff + 64, fo, mo * P:(mo + 1) * P],
                            rhs=qtv[poff:poff + 64, fo, :],
                            start=True, stop=True,
                        )
                    nc.scalar.activation(
                        out=ET[:, g * 1024:(g + 1) * 1024],
                        in_=ps_sc, func=AF.Exp, scale=0.125,
                    )
                    # attended for the 2 fresh m-chunks
                    for j in range(2):
                        mo = 2 * g + j
                        nc.tensor.matmul(
                            ps_at,
                            lhsT=v4v[:, mo, h, 0:65],
                            rhs=etv[:, mo, :],
                            start=(mo == 0), stop=(mo == MO - 1),
                            skip_group_check=True,
                        )
                # epilogue: r, attnT, avg accumulation
                rsum = att_sb.tile([1, 512], f32, tag="rsum", bufs=2,
                                   name=f"rsum_{h}")
                nc.vector.reciprocal(rsum, ps_at[64:65, :])
                rbf = att_sb.tile([1, 1024], bf16, tag="rbf", bufs=2,
                                  name=f"rbf_{h}")
                nc.gpsimd.tensor_copy(rbf[:, 0:512], rsum)
                nc.gpsimd.tensor_copy(rbf[:, 512:1024], rsum)
                ps_r = ps_att_pool.tile([P, 1024], f32, tag="rrep", bufs=1,
                                        name=f"ps_r_{h}")
                nc.tensor.matmul(ps_r[:, 0:512], lhsT=ones_r128,
                                 rhs=rbf[:, 0:512], start=True, stop=True)
                nc.tensor.matmul(ps_r[:, 512:1024], lhsT=ones_r128,
                                 rhs=rbf[:, 512:1024], start=True, stop=True)
                r_rep = att_sb.tile([P, 1024], f16, tag="rrep_sb", bufs=2,
                                    name=f"r_rep_{h}")
                nc.scalar.copy(r_rep, ps_r)
                nc.vector.tensor_mul(
                    atv[poff:poff + 64, fo, :], ps_at[0:64, :],
                    r_rep[0:64, 0:512])
                # avg^T += E^T * r (f16 acc); multiplies first to free ET
                tmps = []
                for mp in range(MO // 2):
                    eng = nc.gpsimd if mp == 3 else nc.vector
                    tmp = att_sb.tile([P, 1024], f16, tag="tmp", bufs=4,
                                      name=f"tmp_{h}_{mp}")
                    eng.tensor_mul(tmp, ET[:, mp * 1024:(mp + 1) * 1024], r_rep)
                    tmps.append(tmp)
                for mp in range(MO // 2):
                    eng = nc.gpsimd if mp == 3 else nc.vector
                    if h == 0:
                        eng.tensor_copy(avg[:, mp * 1024:(mp + 1) * 1024],
                                        tmps[mp])
                    else:
                        eng.tensor_add(avg[:, mp * 1024:(mp + 1) * 1024],
                                       avg[:, mp * 1024:(mp + 1) * 1024],
                                       tmps[mp])

        # ================= phase 3: output projection + LayerNorm =================
        wotv = WoT.rearrange("p (ei f) -> p ei f", ei=EI)
        yT = _tc_tile([P, FO * SLOC], f32, name="yT")
        ytv = yT.rearrange("p (fo s) -> p fo s", fo=FO)

        with (
            tc.tile_pool(name="ph3_sb", bufs=1) as ph3_sb,
            tc.tile_pool(name="ps3", bufs=1, space="PSUM") as ps3,
        ):
            # y^T = WoT.T @ attnT + bo
            for fo in range(FO):
                ps = ps3.tile([P, 512], f32, tag="y", bufs=2, name=f"ps_y_{fo}")
                for ei in range(EI):
                    nc.tensor.matmul(
                        ps,
                        lhsT=wotv[:, ei, fo * P:(fo + 1) * P],
                        rhs=atv[:, ei, :],
                        start=(ei == 0), stop=False,
                    )
                nc.tensor.matmul(
                    ps, lhsT=bias_sb["bo"][:, fo * P:(fo + 1) * P],
                    rhs=ones_r512, start=False, stop=True,
                )
                eng = nc.vector if fo % 2 == 0 else nc.scalar
                if eng is nc.vector:
                    nc.vector.tensor_copy(ytv[:, fo, :], ps)
                else:
                    nc.scalar.copy(ytv[:, fo, :], ps)

        # transpose y^T -> y [s, e]; + residual; LayerNorm; store
        with (
            tc.tile_pool(name="ph4_sb", bufs=1) as ph4_sb,
            tc.tile_pool(name="ps4", bufs=1, space="PSUM") as ps4,
        ):
            for so in range(SG):
                ps_tr = ps4.tile([P, E], f32, tag="ytr", bufs=2, name=f"ps_ytr_{so}")
                for fo in range(FO):
                    nc.tensor.transpose(
                        ps_tr[:, fo * P:(fo + 1) * P],
                        ytv[:, fo, so * P:(so + 1) * P],
                        ident_f32,
                    )
                q_res = ph4_sb.tile([P, E], f32, tag="qres", bufs=2,
                                    name=f"q_res_{so}")
                nc.sync.dma_start(out=q_res, in_=dq[so * P:(so + 1) * P, :])
                y_sb = ph4_sb.tile([P, E], f32, tag="y", bufs=2, name=f"y_{so}")
                nc.vector.tensor_add(y_sb, ps_tr, q_res)
                # LayerNorm stats
                stats = ph4_sb.tile([P, 2, 6], f32, tag="stats", bufs=2,
                                    name=f"stats_{so}")
                nc.vector.bn_stats(stats[:, 0, :], y_sb[:, 0:512])
                nc.vector.bn_stats(stats[:, 1, :], y_sb[:, 512:1024])
                mv = ph4_sb.tile([P, 2], f32, tag="mv", bufs=2, name=f"mv_{so}")
                nc.vector.bn_aggr(mv, stats)
                # rstd = exp(-0.5*ln(var+eps))
                lnv = ph4_sb.tile([P, 1], f32, tag="lnv", bufs=2, name=f"lnv_{so}")
                nc.scalar.activation(lnv, mv[:, 1:2], AF.Ln, bias=eps_sb)
                rstd = ph4_sb.tile([P, 1], f32, tag="rstd", bufs=2,
                                   name=f"rstd_{so}")
                nc.scalar.activation(rstd, lnv, AF.Exp, scale=-0.5)
                # (y - mean) * rstd, then gamma/beta
                nc.vector.tensor_scalar(
                    out=y_sb, in0=y_sb, scalar1=mv[:, 0:1], scalar2=rstd,
                    op0=mybir.AluOpType.subtract, op1=mybir.AluOpType.mult,
                )
                nc.vector.tensor_mul(y_sb, y_sb, gamma_rep)
                nc.vector.tensor_add(y_sb, y_sb, beta_rep)
                nc.sync.dma_start(out=dout[so * P:(so + 1) * P, :], in_=y_sb)

            # avg^T -> avg [s, m] (transpose, scale 1/16, store)
            for so in range(SG):
                av_out = ph4_sb.tile([P, M], f32, tag="avout", bufs=2,
                                     name=f"av_out_{so}")
                for mg in range(MO // 4):
                    ps_av = ps4.tile([P, 512], f16, tag="avtr", bufs=2,
                                     name=f"ps_av_{so}_{mg}")
                    for j in range(4):
                        mo = mg * 4 + j
                        nc.tensor.transpose(
                            ps_av[:, j * P:(j + 1) * P],
                            avgv[:, mo, so * P:(so + 1) * P],
                            ident_f16,
                        )
                    nc.scalar.mul(av_out[:, mg * 512:(mg + 1) * 512], ps_av,
                                  1.0 / H)
                nc.sync.dma_start(out=davg[so * P:(so + 1) * P, :], in_=av_out)

        for fr in reversed(_frees):
            fr()

    nc.compile()
    return nc


def _get_nc():
    if "nc" not in _CACHE:
        _CACHE["nc"] = _build_nc()
    return _CACHE["nc"]


def _make_in_maps(inputs):
    query = np.ascontiguousarray(np.asarray(inputs["query"], dtype=np.float32))
    keys = np.ascontiguousarray(np.asarray(inputs["keys"], dtype=np.float32))
    values = np.ascontiguousarray(np.asarray(inputs["values"], dtype=np.float32))
    fixed = {
        "wq": np.ascontiguousarray(np.asarray(inputs["Wq"], np.float32)),
        "wk": np.ascontiguousarray(np.asarray(inputs["Wk"], np.float32)),
        "wv": np.ascontiguousarray(np.asarray(inputs["Wv"], np.float32)),
        "wo": np.ascontiguousarray(np.asarray(inputs["Wo"], np.float32)),
        "bq": np.ascontiguousarray(np.asarray(inputs["bq"], np.float32)),
        "bk": np.ascontiguousarray(np.asarray(inputs["bk"], np.float32)),
        "bv": np.ascontiguousarray(np.asarray(inputs["bv"], np.float32)),
        "bo": np.ascontiguousarray(np.asarray(inputs["bo"], np.float32)),
        "gamma": np.ascontiguousarray(np.asarray(inputs["gamma"], np.float32)),
        "beta": np.ascontiguousarray(np.asarray(inputs["beta"], np.float32)),
    }
    in_maps = []
    for c in range(NCORES):
        b, r = divmod(c, GROUP)
        m = dict(fixed)
        m["q"] = np.ascontiguousarray(query[b, r * SLOC:(r + 1) * SLOC, :])
        m["keys"] = np.ascontiguousarray(keys[b, r * SLOC:(r + 1) * SLOC, :])
        m["values"] = np.ascontiguousarray(values[b, r * SLOC:(r + 1) * SLOC, :])
        in_maps.append(m)
    return in_maps


def _run(inputs, trace=False):
    from concourse.bass_utils import run_bass_kernel_spmd

    nc = _get_nc()
    in_maps = _make_in_maps(inputs)
    res = run_bass_kernel_spmd(
        nc, in_maps, core_ids=list(range(NCORES)), trace=trace
    )
    out = np.empty((B, S, E), dtype=np.float32)
    avg = np.empty((B, S, M), dtype=np.float32)
    for c in range(NCORES):
        b, r = divmod(c, GROUP)
        out[b, r * SLOC:(r + 1) * SLOC, :] = res.results[c]["out_slice"]
        avg[b, r * SLOC:(r + 1) * SLOC, :] = res.results[c]["avg_slice"]
    return (out, avg), res


def kernel(**inputs):
    (out, avg), _ = _run(inputs, trace=False)
    return out, avg


# revision 40
# speedup vs baseline: 1.0429x; 1.0429x over previous
"""Trainium2 Bass kernel: attention block (B=2, S=M=2048, E=1024, H=16, D=64) + LayerNorm.

Sharding: 8 cores = 2 batches x 4 query-slices of 512 rows each. Each core
computes QKV projections (K/V for the full 2048 keys of its batch), all 16
heads of attention for its 512 query rows, the output projection + residual +
LayerNorm, and the head-mean attention map for its rows. Outputs are
concatenated on the host -- no cross-core communication needed.

Device dataflow ("transposed" orientation, everything bf16 into fp32 PSUM):
  keys^T/values^T/query^T via PE matmul-transpose of cast-DMA'd bf16 chunks
  K^T[f,m] = WkT.T @ keysT      (bias via K=1 matmul)
  V[m,f]   = valuesT.T @ WvT    (+ ones column per head for softmax row-sums)
  Q^T[f,s] = WqT.T @ queryT
  scores^T[m,s] = KT_h.T @ QT_h  -> exp (scale=1/8 folded in) -> E^T bf16
  attended^T[d,s] (+rowsum row) = V_aug.T @ E^T, accumulated over m-chunks
  r = 1/rowsum; attnT = attended^T * r;  avg^T += E^T * r  (per head)
  y^T = WoT.T @ attnT (+bo);  y = transpose(y^T) + query;  LayerNorm; out
  avg = transpose(avg^T) / 16
"""

import numpy as np

B, S, M, E, H = 2, 2048, 2048, 1024, 16
D = E // H
NCORES = 8
GROUP = 4
SLOC = S // GROUP  # 512 query rows per core
P = 128
EI = E // P   # 8  contraction chunks over features
FO = E // P   # 8  output-feature chunks
MO = M // P   # 16 key chunks
SG = SLOC // P  # 4 s-chunks
VSTRIDE = 66  # per-head stride in V tile: 64 values + 1 ones + 1 pad
LN_EPS = 1e-5

_CACHE = {}


def _build_nc():
    import concourse.bass as bass
    import concourse.tile as tile
    from concourse import bacc, mybir
    from concourse.masks import make_identity

    f32 = mybir.dt.float32
    bf16 = mybir.dt.bfloat16
    f16 = mybir.dt.float16
    AF = mybir.ActivationFunctionType

    nc = bacc.Bacc(
        "TRN2", target_bir_lowering=False, debug=False, num_devices=NCORES
    )

    dq = nc.dram_tensor("q", [SLOC, E], f32, kind="ExternalInput").ap()
    dkeys = nc.dram_tensor("keys", [SLOC, E], f32, kind="ExternalInput").ap()
    dvalues = nc.dram_tensor("values", [SLOC, E], f32, kind="ExternalInput").ap()
    # DRAM staging for the K/V AllGather (4-way tensor parallel per batch).
    # K^T slice and V slice ride in ONE gather (both 2-byte dtypes).
    KCOLS = FO * SLOC            # 4096
    VCOLS = 4 * 16 * VSTRIDE     # 4224
    kt_stage = nc.dram_tensor("kt_stage", [P * KCOLS], bf16).ap()
    kt_gath = nc.dram_tensor("kt_gath", [GROUP * P * KCOLS], bf16).ap()
    v_stage = nc.dram_tensor("v_stage", [P * VCOLS], bf16).ap()
    v_gath = nc.dram_tensor("v_gath", [GROUP * P * VCOLS], bf16).ap()
    dwq = nc.dram_tensor("wq", [E, E], f32, kind="ExternalInput").ap()
    dwk = nc.dram_tensor("wk", [E, E], f32, kind="ExternalInput").ap()
    dwv = nc.dram_tensor("wv", [E, E], f32, kind="ExternalInput").ap()
    dwo = nc.dram_tensor("wo", [E, E], f32, kind="ExternalInput").ap()
    dbq = nc.dram_tensor("bq", [E], f32, kind="ExternalInput").ap()
    dbk = nc.dram_tensor("bk", [E], f32, kind="ExternalInput").ap()
    dbv = nc.dram_tensor("bv", [E], f32, kind="ExternalInput").ap()
    dbo = nc.dram_tensor("bo", [E], f32, kind="ExternalInput").ap()
    dgamma = nc.dram_tensor("gamma", [E], f32, kind="ExternalInput").ap()
    dbeta = nc.dram_tensor("beta", [E], f32, kind="ExternalInput").ap()
    dout = nc.dram_tensor("out_slice", [SLOC, E], f32, kind="ExternalOutput").ap()
    davg = nc.dram_tensor("avg_slice", [SLOC, M], f32, kind="ExternalOutput").ap()

    with tile.TileContext(nc) as tc:
        _frees = []

        def _tc_tile(shape, dtype, name):
            t, fr = tc.tile(shape, dtype, name=name)
            _frees.append(fr)
            return t

        # ---------------- constants ----------------
        ident_bf = _tc_tile([P, P], bf16, name="ident_bf")
        make_identity(nc, ident_bf)
        ident_f32 = _tc_tile([P, P], f32, name="ident_f32")
        make_identity(nc, ident_f32)
        ident_f16 = _tc_tile([P, P], f16, name="ident_f16")
        make_identity(nc, ident_f16)
        ones_r512 = _tc_tile([1, 512], bf16, name="ones_r512")
        nc.vector.memset(ones_r512, 1.0)
        ones_r128 = _tc_tile([1, P], bf16, name="ones_r128")
        nc.vector.memset(ones_r128, 1.0)
        ones_f128 = _tc_tile([1, P], f32, name="ones_f128")
        nc.vector.memset(ones_f128, 1.0)
        eps_sb = _tc_tile([P, 1], f32, name="eps_sb")
        nc.vector.memset(eps_sb, LN_EPS)

        # biases as [1, E] bf16 (loaded later, after the first weight chunks)
        bias_sb = {}
        for nm in ("bq", "bk", "bv", "bo"):
            bias_sb[nm] = _tc_tile([1, E], bf16, name=f"{nm}_sb")

        # ---------------- persistent big tiles ----------------
        KT = _tc_tile([P, FO * M], bf16, name="KT")        # [f, m]
        V4 = _tc_tile([P, MO * 16 * VSTRIDE], f16, name="V4")   # [m, h*66]
        QT = _tc_tile([P, FO * SLOC], bf16, name="QT")     # [f, s]
        WoT = _tc_tile([P, EI * E], bf16, name="WoT")     # [e_in, f]
        avg = _tc_tile([P, MO * SLOC], f16, name="avg")   # [m, s] f16 accumulator
        attnT = _tc_tile([P, EI * SLOC], bf16, name="attnT")  # [e, s]
        gamma_rep = _tc_tile([P, E], f32, name="gamma_rep")
        beta_rep = _tc_tile([P, E], f32, name="beta_rep")

        v4v = V4.rearrange("p (mi h c) -> p mi h c", mi=MO, h=16)

        # ================= phase 0/1: transposes + projections =================
        with (
            tc.tile_pool(name="ph01", bufs=1) as ph01,
            tc.tile_pool(name="stage", bufs=8) as stage,
            tc.tile_pool(name="ps01", bufs=1, space="PSUM") as ps01,
        ):
            inT = None  # [128, 16384] bf16, reused for keysT/valuesT
            wT = None   # [128, 8192] bf16, reused for WkT/WvT/WqT

            def load_transposed(dsrc, nrows, dst_tag, ident):
                """DMA [nrows, E] f32 -> bf16 chunks, PE-transpose into a
                [128, EI*nrows] bf16 tile (e on partitions)."""
                dstT = ph01.tile([P, EI * nrows], bf16, tag=dst_tag, name=dst_tag)
                nch = nrows // P
                for mi in range(nch):
                    st = stage.tile([P, E], bf16, tag="stage", name=f"st_{dst_tag}_{mi}")
                    nc.gpsimd.dma_start(out=st, in_=dsrc[mi * P:(mi + 1) * P, :])
                    pst = ps01.tile([P, E], bf16, tag="tr", bufs=3,
                                    name=f"pst_{dst_tag}_{mi}")
                    for ei in range(EI):
                        nc.tensor.transpose(
                            pst[:, ei * P:(ei + 1) * P],
                            st[:, ei * P:(ei + 1) * P],
                            ident,
                        )
                    # strided copy: block ei -> dstT cols ei*nrows + mi*128
                    dv = dstT.rearrange("p (ei m) -> p ei m", ei=EI)
                    eng = nc.vector if mi % 2 == 0 else nc.scalar
                    if eng is nc.vector:
                        nc.vector.tensor_copy(
                            dv[:, :, mi * P:(mi + 1) * P],
                            pst.rearrange("p (ei m) -> p ei m", ei=EI),
                        )
                    else:
                        nc.scalar.copy(
                            dv[:, :, mi * P:(mi + 1) * P],
                            pst.rearrange("p (ei m) -> p ei m", ei=EI),
                        )
                return dstT

            def load_w_transposed(dsrc, nm):
                wt = ph01.tile([P, EI * E], bf16, tag="wT", name=nm)
                wv_ = wt.rearrange("p (ei f) -> p ei f", ei=EI)
                for fi in range(FO):
                    st = stage.tile([P, E], bf16, tag="stage", name=f"st_{nm}_{fi}")
                    nc.gpsimd.dma_start(out=st, in_=dsrc[fi * P:(fi + 1) * P, :])
                    pst = ps01.tile([P, E], bf16, tag="tr", bufs=3,
                                    name=f"pst_{nm}_{fi}")
                    for ei in range(EI):
                        nc.tensor.transpose(
                            pst[:, ei * P:(ei + 1) * P],
                            st[:, ei * P:(ei + 1) * P],
                            ident_bf,
                        )
                    eng = nc.vector if fi % 2 == 0 else nc.scalar
                    if eng is nc.vector:
                        nc.vector.tensor_copy(
                            wv_[:, :, fi * P:(fi + 1) * P],
                            pst.rearrange("p (ei f) -> p ei f", ei=EI),
                        )
                    else:
                        nc.scalar.copy(
                            wv_[:, :, fi * P:(fi + 1) * P],
                            pst.rearrange("p (ei f) -> p ei f", ei=EI),
                        )
                return wt

            # ---- K^T slice = Wk @ keys_slice^T, then AllGather over group ----
            wT = load_w_transposed(dwk, "WkT")
            for nm, dap in (("bk", dbk), ("bv", dbv), ("bq", dbq), ("bo", dbo)):
                nc.gpsimd.dma_start(
                    out=bias_sb[nm],
                    in_=bass.AP(dap.tensor, 0, [[E, 1], [1, E]]))
            inT = load_transposed(dkeys, SLOC, "inT", ident_bf)
            wv_ = wT.rearrange("p (ei f) -> p ei f", ei=EI)
            inv = inT.rearrange("p (ei m) -> p ei m", ei=EI)
            ktv = KT.rearrange("p (fo m) -> p fo m", fo=FO)
            ktp = ph01.tile([P, FO * SLOC], bf16, tag="ktp", name="ktp")
            ktpv = ktp.rearrange("p (fo m) -> p fo m", fo=FO)
            for fo in range(FO):
                ps = ps01.tile([P, 512], f32, tag=f"proj{fo % 4}", bufs=1,
                               name=f"ps_kt_{fo}")
                for ei in range(EI):
                    nc.tensor.matmul(
                        ps,
                        lhsT=wv_[:, ei, fo * P:(fo + 1) * P],
                        rhs=inv[:, ei, :],
                        start=(ei == 0), stop=False,
                    )
                nc.tensor.matmul(
                    ps, lhsT=bias_sb["bk"][:, fo * P:(fo + 1) * P],
                    rhs=ones_r512, start=False, stop=True,
                )
                eng = nc.vector if fo % 2 == 0 else nc.scalar
                if eng is nc.vector:
                    nc.vector.tensor_copy(ktpv[:, fo, :], ps)
                else:
                    nc.scalar.copy(ktpv[:, fo, :], ps)
            nc.sync.dma_start(
                out=kt_stage.rearrange("(p n) -> p n", p=P), in_=ktp)
            nc.gpsimd.collective_compute(
                "AllGather", mybir.AluOpType.bypass,
                replica_groups=[[0, 1, 2, 3], [4, 5, 6, 7]],
                ins=[kt_stage], outs=[kt_gath],
            )


            # ---- V slice = values_slice @ Wv^T, then AllGather ----
            wT = load_w_transposed(dwv, "WvT")  # reuses wT slot
            inT = load_transposed(dvalues, SLOC, "inT", ident_bf)
            wv_ = wT.rearrange("p (ei f) -> p ei f", ei=EI)
            inv = inT.rearrange("p (ei m) -> p ei m", ei=EI)
            vp = ph01.tile([P, 4 * 16 * VSTRIDE], f16, tag="vp", name="vp")
            vpv = vp.rearrange("p (mi h c) -> p mi h c", mi=4, h=16)
            nc.vector.memset(vpv[:, :, :, 64:65], 1.0)
            for mi in range(4):
                for half in range(2):
                    ps = ps01.tile([P, 512], f32, tag=f"proj{(2 * mi + half) % 4}",
                                   bufs=1, name=f"ps_v_{mi}_{half}")
                    for ei in range(EI):
                        nc.tensor.matmul(
                            ps,
                            lhsT=inv[:, ei, mi * P:(mi + 1) * P],
                            rhs=wv_[:, ei, half * 512:(half + 1) * 512],
                            start=(ei == 0), stop=False,
                        )
                    nc.tensor.matmul(
                        ps, lhsT=ones_r128,
                        rhs=bias_sb["bv"][:, half * 512:(half + 1) * 512],
                        start=False, stop=True,
                    )
                    psv = ps.rearrange("p (h c) -> p h c", h=8)
                    eng = nc.vector if (mi * 2 + half) % 2 == 0 else nc.scalar
                    if eng is nc.vector:
                        nc.vector.tensor_copy(
                            vpv[:, mi, half * 8:(half + 1) * 8, 0:64], psv)
                    else:
                        nc.scalar.copy(
                            vpv[:, mi, half * 8:(half + 1) * 8, 0:64], psv)
            nc.sync.dma_start(
                out=v_stage.rearrange("(p n) -> p n", p=P),
                in_=vp.bitcast(bf16))
            nc.gpsimd.collective_compute(
                "AllGather", mybir.AluOpType.bypass,
                replica_groups=[[0, 1, 2, 3], [4, 5, 6, 7]],
                ins=[v_stage], outs=[v_gath],
            )


            # ---- Q^T = Wq @ query^T ----
            wT = load_w_transposed(dwq, "WqT")
            wv_ = wT.rearrange("p (ei f) -> p ei f", ei=EI)
            qT = ph01.tile([P, EI * SLOC], bf16, tag="qT", name="queryT")
            qv = qT.rearrange("p (ei s) -> p ei s", ei=EI)
            for mi in range(SG):
                st = stage.tile([P, E], bf16, tag="stage", name=f"st_q_{mi}")
                nc.gpsimd.dma_start(out=st, in_=dq[mi * P:(mi + 1) * P, :])
                pst = ps01.tile([P, E], bf16, tag="tr", bufs=3, name=f"pst_q_{mi}")
                for ei in range(EI):
                    nc.tensor.transpose(
                        pst[:, ei * P:(ei + 1) * P],
                        st[:, ei * P:(ei + 1) * P],
                        ident_bf,
                    )
                nc.vector.tensor_copy(
                    qv[:, :, mi * P:(mi + 1) * P],
                    pst.rearrange("p (ei s) -> p ei s", ei=EI),
                )
            qtv = QT.rearrange("p (fo s) -> p fo s", fo=FO)
            for fo in range(FO):
                ps = ps01.tile([P, 512], f32, tag=f"proj{fo % 4}", bufs=1,
                               name=f"ps_qt_{fo}")
                for ei in range(EI):
                    nc.tensor.matmul(
                        ps,
                        lhsT=wv_[:, ei, fo * P:(fo + 1) * P],
                        rhs=qv[:, ei, :],
                        start=(ei == 0), stop=False,
                    )
                nc.tensor.matmul(
                    ps, lhsT=bias_sb["bq"][:, fo * P:(fo + 1) * P],
                    rhs=ones_r512, start=False, stop=True,
                )
                eng = nc.vector if fo % 2 == 0 else nc.scalar
                if eng is nc.vector:
                    nc.vector.tensor_copy(qtv[:, fo, :], ps)
                else:
                    nc.scalar.copy(qtv[:, fo, :], ps)

            # ---- WoT (kept for output projection) ----
            wt_o = load_w_transposed(dwo, "WoT_tmp")
            nc.vector.tensor_copy(WoT, wt_o)

            # ---- replicate gamma/beta across partitions (overlaps gather) ----
            gb_raw = ph01.tile([1, 2 * E], f32, tag="gb", name="gb_raw")
            nc.sync.dma_start(out=gb_raw[:, 0:E],
                              in_=bass.AP(dgamma.tensor, 0, [[E, 1], [1, E]]))
            nc.sync.dma_start(out=gb_raw[:, E:2 * E],
                              in_=bass.AP(dbeta.tensor, 0, [[E, 1], [1, E]]))
            for i, dst in ((0, gamma_rep), (1, beta_rep)):
                for j in range(2):
                    ps = ps01.tile([P, 512], f32, tag=f"proj{j}", bufs=1,
                                   name=f"ps_g_{i}_{j}")
                    nc.tensor.matmul(
                        ps, lhsT=ones_f128,
                        rhs=gb_raw[:, i * E + j * 512: i * E + (j + 1) * 512],
                        start=True, stop=True,
                    )
                    nc.vector.tensor_copy(dst[:, j * 512:(j + 1) * 512], ps)

        # gather -> SBUF loads (outside ph01 so its pools release early)
        ktg3 = kt_gath.rearrange("(r p fo m) -> r p fo m", r=GROUP, p=P, fo=FO)
        for r in range(GROUP):
            nc.sync.dma_start(
                out=ktv[:, :, r * SLOC:(r + 1) * SLOC], in_=ktg3[r])
        vg = v_gath.rearrange("(r p n) -> r p n", r=GROUP, p=P)
        v4f = V4.rearrange("p (mi n) -> p mi n", mi=GROUP)
        for r in range(GROUP):
            nc.sync.dma_start(out=v4f[:, r, :], in_=vg[r].bitcast(f16))

        # ================= phase 2: attention =================
        with (
            tc.tile_pool(name="att_sb", bufs=1) as att_sb,
            tc.tile_pool(name="ps_att", bufs=1, space="PSUM") as ps_att_pool,
        ):
            avgv = avg.rearrange("p (mo s) -> p mo s", mo=MO)
            atv = attnT.rearrange("p (ei s) -> p ei s", ei=EI)
            for h in range(H):
                fo = h // 2
                poff = (h % 2) * 64
                ET = att_sb.tile([P, MO * SLOC], f16, tag="ET", bufs=3,
                                 name=f"ET_{h}")
                etv = ET.rearrange("p (mo s) -> p mo s", mo=MO)
                ps_at = ps_att_pool.tile([65, 512], f32, tag="att", bufs=2,
                                         name=f"ps_at_{h}")
                # scores^T in groups of 2 m-chunks (double-buffered psum)
                for g in range(MO // 2):
                    ps_sc = ps_att_pool.tile([P, 1024], f32, tag="sc", bufs=2,
                                             name=f"ps_sc_{h}_{g}")
                    for j in range(2):
                        mo = 2 * g + j
                        nc.tensor.matmul(
                            ps_sc[:, j * 512:(j + 1) * 512],
                            lhsT=ktv[poff:poff + 64, fo, mo * P:(mo + 1) * P],
                            rhs=qtv[poff:poff + 64, fo, :],
                            start=True, stop=True,
                        )
                    nc.scalar.activation(
                        out=ET[:, g * 1024:(g + 1) * 1024],
                        in_=ps_sc, func=AF.Exp, scale=0.125,
                    )
                    # attended for the 2 fresh m-chunks
                    for j in range(2):
                        mo = 2 * g + j
                        nc.tensor.matmul(
                            ps_at,
                            lhsT=v4v[:, mo, h, 0:65],
                            rhs=etv[:, mo, :],
                            start=(mo == 0), stop=(mo == MO - 1),
                            skip_group_check=True,
                        )
                # epilogue: r, attnT, avg accumulation
                rsum = att_sb.tile([1, 512], f32, tag="rsum", bufs=2,
                                   name=f"rsum_{h}")
                nc.vector.reciprocal(rsum, ps_at[64:65, :])
                rbf = att_sb.tile([1, 1024], bf16, tag="rbf", bufs=2,
                                  name=f"rbf_{h}")
                nc.gpsimd.tensor_copy(rbf[:, 0:512], rsum)
                nc.gpsimd.tensor_copy(rbf[:, 512:1024], rsum)
                ps_r = ps_att_pool.tile([P, 1024], f32, tag="rrep", bufs=1,
                                        name=f"ps_r_{h}")
                nc.tensor.matmul(ps_r[:, 0:512], lhsT=ones_r128,
                                 rhs=rbf[:, 0:512], start=True, stop=True)
                nc.tensor.matmul(ps_r[:, 512:1024], lhsT=ones_r128,
                                 rhs=rbf[:, 512:1024], start=True, stop=True)
                r_rep = att_sb.tile([P, 1024], f16, tag="rrep_sb", bufs=2,
                                    name=f"r_rep_{h}")
                nc.scalar.copy(r_rep, ps_r)
                nc.vector.tensor_mul(
                    atv[poff:poff + 64, fo, :], ps_at[0:64, :],
                    r_rep[0:64, 0:512])
                # avg^T += E^T * r (f16 acc); multiplies first to free ET
                tmps = []
                for mp in range(MO // 2):
                    eng = nc.gpsimd if mp == 3 else nc.vector
                    tmp = att_sb.tile([P, 1024], f16, tag="tmp", bufs=4,
                                      name=f"tmp_{h}_{mp}")
                    eng.tensor_mul(tmp, ET[:, mp * 1024:(mp + 1) * 1024], r_rep)
                    tmps.append(tmp)
                for mp in range(MO // 2):
                    eng = nc.gpsimd if mp == 3 else nc.vector
                    if h == 0:
                        eng.tensor_copy(avg[:, mp * 1024:(mp + 1) * 1024],
                                        tmps[mp])
                    else:
                        eng.tensor_add(avg[:, mp * 1024:(mp + 1) * 1024],
                                       avg[:, mp * 1024:(mp + 1) * 1024],
                                       tmps[mp])

        # ================= phase 3: output projection + LayerNorm =================
        wotv = WoT.rearrange("p (ei f) -> p ei f", ei=EI)
        yT = _tc_tile([P, FO * SLOC], f32, name="yT")
        ytv = yT.rearrange("p (fo s) -> p fo s", fo=FO)

        with (
            tc.tile_pool(name="ph3_sb", bufs=1) as ph3_sb,
            tc.tile_pool(name="ps3", bufs=1, space="PSUM") as ps3,
        ):
            # y^T = WoT.T @ attnT + bo
            for fo in range(FO):
                ps = ps3.tile([P, 512], f32, tag="y", bufs=2, name=f"ps_y_{fo}")
                for ei in range(EI):
                    nc.tensor.matmul(
                        ps,
                        lhsT=wotv[:, ei, fo * P:(fo + 1) * P],
                        rhs=atv[:, ei, :],
                        start=(ei == 0), stop=False,
                    )
                nc.tensor.matmul(
                    ps, lhsT=bias_sb["bo"][:, fo * P:(fo + 1) * P],
                    rhs=ones_r512, start=False, stop=True,
                )
                eng = nc.vector if fo % 2 == 0 else nc.scalar
                if eng is nc.vector:
                    nc.vector.tensor_copy(ytv[:, fo, :], ps)
                else:
                    nc.scalar.copy(ytv[:, fo, :], ps)

        # transpose y^T -> y [s, e]; + residual; LayerNorm; store
        with (
            tc.tile_pool(name="ph4_sb", bufs=1) as ph4_sb,
            tc.tile_pool(name="ps4", bufs=1, space="PSUM") as ps4,
        ):
            for so in range(SG):
                ps_tr = ps4.tile([P, E], f32, tag="ytr", bufs=2, name=f"ps_ytr_{so}")
                for fo in range(FO):
                    nc.tensor.transpose(
                        ps_tr[:, fo * P:(fo + 1) * P],
                        ytv[:, fo, so * P:(so + 1) * P],
                        ident_f32,
                    )
                q_res = ph4_sb.tile([P, E], f32, tag="qres", bufs=2,
                                    name=f"q_res_{so}")
                nc.sync.dma_start(out=q_res, in_=dq[so * P:(so + 1) * P, :])
                y_sb = ph4_sb.tile([P, E], f32, tag="y", bufs=2, name=f"y_{so}")
                nc.vector.tensor_add(y_sb, ps_tr, q_res)
                # LayerNorm stats
                stats = ph4_sb.tile([P, 2, 6], f32, tag="stats", bufs=2,
                                    name=f"stats_{so}")
                nc.vector.bn_stats(stats[:, 0, :], y_sb[:, 0:512])
                nc.vector.bn_stats(stats[:, 1, :], y_sb[:, 512:1024])
                mv = ph4_sb.tile([P, 2], f32, tag="mv", bufs=2, name=f"mv_{so}")
                nc.vector.bn_aggr(mv, stats)
                # rstd = exp(-0.5*ln(var+eps))
                lnv = ph4_sb.tile([P, 1], f32, tag="lnv", bufs=2, name=f"lnv_{so}")
                nc.scalar.activation(lnv, mv[:, 1:2], AF.Ln, bias=eps_sb)
                rstd = ph4_sb.tile([P, 1], f32, tag="rstd", bufs=2,
                                   name=f"rstd_{so}")
                nc.scalar.activation(rstd, lnv, AF.Exp, scale=-0.5)
                # (y - mean) * rstd, then gamma/beta
                nc.vector.tensor_scalar(
                    out=y_sb, in0=y_sb, scalar1=mv[:, 0:1], scalar2=rstd,
                    op0=mybir.AluOpType.subtract, op1=mybir.AluOpType.mult,
                )
                nc.vector.tensor_mul(y_sb, y_sb, gamma_rep)
                nc.vector.tensor_add(y_sb, y_sb, beta_rep)
                nc.sync.dma_start(out=dout[so * P:(so + 1) * P, :], in_=y_sb)

            # avg^T -> avg [s, m] (transpose, scale 1/16, store)
            for so in range(SG):
                av_out = ph4_sb.tile([P, M], f32, tag="avout", bufs=2,
                                     name=f"av_out_{so}")
                for mg in range(MO // 4):
                    ps_av = ps4.tile([P, 512], f16, tag="avtr", bufs=2,
                                     name=f"ps_av_{so}_{mg}")
                    for j in range(4):
                        mo = mg * 4 + j
                        nc.tensor.transpose(
                            ps_av[:, j * P:(j + 1) * P],
                            avgv[:, mo, so * P:(so + 1) * P],
                            ident_f16,
                        )
                    nc.scalar.mul(av_out[:, mg * 512:(mg + 1) * 512], ps_av,
                                  1.0 / H)
                nc.sync.dma_start(out=davg[so * P:(so + 1) * P, :], in_=av_out)

        for fr in reversed(_frees):
            fr()

    nc.compile()
    return nc


def _get_nc():
    if "nc" not in _CACHE:
        _CACHE["nc"] = _build_nc()
    return _CACHE["nc"]


def _make_in_maps(inputs):
    query = np.ascontiguousarray(np.asarray(inputs["query"], dtype=np.float32))
    keys = np.ascontiguousarray(np.asarray(inputs["keys"], dtype=np.float32))
    values = np.ascontiguousarray(np.asarray(inputs["values"], dtype=np.float32))
    fixed = {
        "wq": np.ascontiguousarray(np.asarray(inputs["Wq"], np.float32)),
        "wk": np.ascontiguousarray(np.asarray(inputs["Wk"], np.float32)),
        "wv": np.ascontiguousarray(np.asarray(inputs["Wv"], np.float32)),
        "wo": np.ascontiguousarray(np.asarray(inputs["Wo"], np.float32)),
        "bq": np.ascontiguousarray(np.asarray(inputs["bq"], np.float32)),
        "bk": np.ascontiguousarray(np.asarray(inputs["bk"], np.float32)),
        "bv": np.ascontiguousarray(np.asarray(inputs["bv"], np.float32)),
        "bo": np.ascontiguousarray(np.asarray(inputs["bo"], np.float32)),
        "gamma": np.ascontiguousarray(np.asarray(inputs["gamma"], np.float32)),
        "beta": np.ascontiguousarray(np.asarray(inputs["beta"], np.float32)),
    }
    in_maps = []
    for c in range(NCORES):
        b, r = divmod(c, GROUP)
        m = dict(fixed)
        m["q"] = np.ascontiguousarray(query[b, r * SLOC:(r + 1) * SLOC, :])
        m["keys"] = np.ascontiguousarray(keys[b, r * SLOC:(r + 1) * SLOC, :])
        m["values"] = np.ascontiguousarray(values[b, r * SLOC:(r + 1) * SLOC, :])
        in_maps.append(m)
    return in_maps


def _run(inputs, trace=False):
    from concourse.bass_utils import run_bass_kernel_spmd

    nc = _get_nc()
    in_maps = _make_in_maps(inputs)
    res = run_bass_kernel_spmd(
        nc, in_maps, core_ids=list(range(NCORES)), trace=trace
    )
    out = np.empty((B, S, E), dtype=np.float32)
    avg = np.empty((B, S, M), dtype=np.float32)
    for c in range(NCORES):
        b, r = divmod(c, GROUP)
        out[b, r * SLOC:(r + 1) * SLOC, :] = res.results[c]["out_slice"]
        avg[b, r * SLOC:(r + 1) * SLOC, :] = res.results[c]["avg_slice"]
    return (out, avg), res


def kernel(**inputs):
    (out, avg), _ = _run(inputs, trace=False)
    return out, avg
